# revision 11
# baseline (speedup 1.0000x reference)
"""Bahdanau attention kernel for Trainium2 (8 NeuronCores, data-parallel over batch).

Computes, for inputs dec_hidden [B,H], enc_output [B,S,H], W1/W2 [H,U], b1/b2 [U],
V [U,1], bV [1]:
    q     = dec_hidden @ W1 + b1                      [B, U]
    k     = enc_output @ W2 + b2                      [B, S, U]
    score = tanh(q[:,None,:] + k) @ V + bV            [B, S, 1]
    w     = softmax(score, axis=1)                    [B, S, 1]
    ctx   = sum(w * enc_output, axis=1)               [B, H]
returns (ctx, w).

Sharding: batch is split 32 -> 8 cores x 4 examples. Weights replicated.
No collectives needed.

Device dataflow per core (4 examples):
  - kT tiles [u=128, s=512] = W2_tile.T @ encT_tile on TensorE (fp16 in, fp32 accum)
  - tanh(kT + (q+b1+b2)[u]) fused on ScalarE (bias = per-partition q column)
  - score row [1, s] = V_chunk.T @ tanh_tile accumulated over u chunks (TensorE)
  - softmax along free dim on 1 partition (exp w/ fused sum on ScalarE, DVE normalize)
  - w transposed to [s=128,1] tiles via TensorE transpose-mode
  - ctx row [1, h] = w_tile.T @ enc_tile accumulated over s chunks (TensorE)

bV is dropped: softmax(x + c) == softmax(x), and w is the only consumer of score.
b1/b2 are folded into the tanh bias.
"""

import numpy as np

import concourse.bass as bass
import concourse.mybir as mybir
import concourse.tile as tile
from concourse import bacc, bass_utils

# Problem shapes (hardcoded per the harness contract).
B, S, H, U = 32, 2048, 1024, 1024
NCORES = 8
BC = B // NCORES          # 4 examples per core
P = 128                   # partitions
HC = H // P               # 8 h-chunks
UC = U // P               # 8 u-chunks
NB = 512                  # free-dim block (one fp32 PSUM bank)
SB = S // NB              # 4 s-blocks
SJ = S // P               # 16 s-chunks of 128 (for w / context)

F32 = mybir.dt.float32
F16 = mybir.dt.float16

_COMPILED = None  # (nc, results cache key) -- build once per process


def build_nc():
    """Build the single-core Bass program (SPMD across 8 cores)."""
    nc = bacc.Bacc(trn_type="TRN2")

    # ---- DRAM I/O (per-core shard) ----
    encT = nc.dram_tensor("encT", [BC, H, S], F16, kind="ExternalInput").ap()
    encN = nc.dram_tensor("encN", [BC, S, H], F16, kind="ExternalInput").ap()
    decT = nc.dram_tensor("decT", [H, BC], F16, kind="ExternalInput").ap()
    W1 = nc.dram_tensor("W1", [H, U], F16, kind="ExternalInput").ap()
    W2 = nc.dram_tensor("W2", [H, U], F16, kind="ExternalInput").ap()
    Vt = nc.dram_tensor("Vt", [P, UC], F16, kind="ExternalInput").ap()
    b12t = nc.dram_tensor("b12t", [P, UC], F32, kind="ExternalInput").ap()
    out_w = nc.dram_tensor("out_w", [BC, S], F32, kind="ExternalOutput").ap()
    out_ctx = nc.dram_tensor("out_ctx", [BC, H], F32, kind="ExternalOutput").ap()

    TANH = mybir.ActivationFunctionType.Tanh
    EXP = mybir.ActivationFunctionType.Exp

    with tile.TileContext(nc) as tc:
        with (
            tc.tile_pool(name="const", bufs=1) as cpool,
            tc.tile_pool(name="encT", bufs=2) as etpool,
            tc.tile_pool(name="encN", bufs=6) as enpool,
            tc.tile_pool(name="tanh", bufs=4) as thpool,
            tc.tile_pool(name="rows", bufs=2) as rpool,
            tc.tile_pool(name="wt", bufs=2) as wtpool,
            tc.tile_pool(name="mmps", bufs=4, space=bass.MemorySpace.PSUM) as mmps,
            tc.tile_pool(name="rowps", bufs=2, space=bass.MemorySpace.PSUM) as rowps,
            tc.tile_pool(name="tps", bufs=2, space=bass.MemorySpace.PSUM) as tpps,
        ):
            # ---- constants ----
            W1sb = cpool.tile([P, HC, U], F16)
            nc.sync.dma_start(W1sb[:], W1.rearrange("(c p) u -> p c u", p=P))
            W2sb = cpool.tile([P, HC, U], F16)
            nc.sync.dma_start(W2sb[:], W2.rearrange("(c p) u -> p c u", p=P))
            Vtsb = cpool.tile([P, UC], F16)
            nc.sync.dma_start(Vtsb[:], Vt)
            b12sb = cpool.tile([P, UC], F32)
            nc.sync.dma_start(b12sb[:], b12t)
            decTsb = cpool.tile([P, HC, BC], F16)
            nc.sync.dma_start(decTsb[:], decT.rearrange("(c p) b -> p c b", p=P))
            ident = cpool.tile([1, 1], F16)
            nc.vector.memset(ident[:], 1.0)

            # ---- qb[u, b] = (dec @ W1)[b, u] + b1[u] + b2[u] ----
            qbsb = cpool.tile([P, UC, BC], F32)
            for uc in range(UC):
                qps = mmps.tile([P, NB], F32, tag="mm")
                for hc in range(HC):
                    nc.tensor.matmul(
                        qps[:, :BC],
                        W1sb[:, hc, uc * P:(uc + 1) * P],
                        decTsb[:, hc, :],
                        start=(hc == 0),
                        stop=(hc == HC - 1),
                    )
                nc.vector.tensor_scalar_add(
                    qbsb[:, uc, :], qps[:, :BC], b12sb[:, uc:uc + 1]
                )

            # Deferred score-matmul emission: keep PE from stalling on ACT tanh
            # by emitting each score MM two main-MM-groups later.
            pending = []

            def drain_one():
                if pending:
                    pending.pop(0)()

            def drain_all():
                while pending:
                    pending.pop(0)()

            enc_tiles = {}   # b -> encT sbuf tile
            score_rows = {}  # b -> scores sbuf row [1, S]
            w_rows = {}      # b -> normalized weights row [1, S]
            wt_tiles = {}    # b -> [P, SJ] f16 transposed weights

            def phase2(b):
                """Main matmul + tanh + score accumulation for example b."""
                et = etpool.tile([P, HC, S], F16, tag="encT")
                for sb in range(SB):
                    nc.sync.dma_start(
                        et[:, :, sb * NB:(sb + 1) * NB],
                        encT[b].rearrange("(c p) s -> p c s", p=P)[
                            :, :, sb * NB:(sb + 1) * NB
                        ],
                    )
                enc_tiles[b] = et
                scores = rpool.tile([1, S], F32, tag="scores")
                score_rows[b] = scores
                for sb in range(SB):
                    sps = rowps.tile([1, NB], F32, tag="row")
                    for uc in range(UC):
                        mps = mmps.tile([P, NB], F32, tag="mm")
                        for hc in range(HC):
                            nc.tensor.matmul(
                                mps[:],
                                W2sb[:, hc, uc * P:(uc + 1) * P],
                                et[:, hc, sb * NB:(sb + 1) * NB],
                                start=(hc == 0),
                                stop=(hc == HC - 1),
                            )
                        tt = thpool.tile([P, NB], F16, tag="tanh")
                        nc.scalar.activation(
                            tt[:], mps[:], TANH, bias=qbsb[:, uc, b:b + 1]
                        )

                        def score_mm(sps=sps, uc=uc, tt=tt, sb=sb, scores=scores):
                            nc.tensor.matmul(
                                sps[:],
                                Vtsb[:, uc:uc + 1],
                                tt[:],
                                start=(uc == 0),
                                stop=(uc == UC - 1),
                            )
                            if uc == UC - 1:
                                nc.vector.tensor_copy(
                                    scores[:, sb * NB:(sb + 1) * NB], sps[:]
                                )

                        pending.append(score_mm)
                        if len(pending) > 2:
                            drain_one()

            def phase3(b):
                """Softmax over s (free dim, 1 partition) + transpose w."""
                drain_all()
                scores = score_rows[b]
                ew = rpool.tile([1, S], F32, tag="ew")
                sum_e = rpool.tile([1, 1], F32, tag="sum")
                nc.scalar.activation(ew[:], scores[:], EXP, accum_out=sum_e[:])
                rec = rpool.tile([1, 1], F32, tag="rec")
                nc.vector.reciprocal(rec[:], sum_e[:])
                wrow = rpool.tile([1, S], F32, tag="w")
                nc.vector.tensor_scalar_mul(wrow[:], ew[:], rec[:, 0:1])
                w_rows[b] = wrow
                nc.sync.dma_start(out_w[b:b + 1, :], wrow[:])
                wrow16 = rpool.tile([1, S], F16, tag="w16")
                nc.vector.tensor_copy(wrow16[:], wrow[:])
                wt = wtpool.tile([P, SJ], F16, tag="wt")
                for j in range(SJ):
                    tp = tpps.tile([P, 1], F16, tag="tp")
                    nc.tensor.transpose(
                        tp[:], wrow16[0:1, j * P:(j + 1) * P], ident[0:1, 0:1]
                    )
                    nc.vector.tensor_copy(wt[:, j:j + 1], tp[:])
                wt_tiles[b] = wt

            def phase4(b):
                """Context: ctx[h] = sum_s w[s] * enc[s, h] via PE."""
                wt = wt_tiles[b]
                cps0 = rowps.tile([1, NB], F32, tag="row")
                cps1 = rowps.tile([1, NB], F32, tag="row")
                for j in range(SJ):
                    ent = enpool.tile([P, H], F16, tag="encN")
                    nc.sync.dma_start(ent[:], encN[b, j * P:(j + 1) * P, :])
                    nc.tensor.matmul(
                        cps0[:], wt[:, j:j + 1], ent[:, 0:NB],
                        start=(j == 0), stop=(j == SJ - 1),
                    )
                    nc.tensor.matmul(
                        cps1[:], wt[:, j:j + 1], ent[:, NB:H],
                        start=(j == 0), stop=(j == SJ - 1),
                    )
                ctxrow = rpool.tile([1, H], F32, tag="ctx")
                nc.vector.tensor_copy(ctxrow[:, 0:NB], cps0[:])
                nc.vector.tensor_copy(ctxrow[:, NB:H], cps1[:])
                nc.sync.dma_start(out_ctx[b:b + 1, :], ctxrow[:])

            # Emission order staggers phases so PE never waits on ACT/DVE.
            phase2(0)
            phase2(1)
            phase3(0)
            phase4(0)
            phase2(2)
            phase3(1)
            phase4(1)
            phase2(3)
            phase3(2)
            phase4(2)
            phase3(3)
            phase4(3)

    nc.compile()
    return nc


def _get_nc():
    global _COMPILED
    if _COMPILED is None:
        _COMPILED = build_nc()
    return _COMPILED


def make_in_maps(dec_hidden, enc_output, W1, b1, W2, b2, V, bV):
    """Host-side sharding / layout prep. Returns list of 8 per-core input maps."""
    dec_hidden = np.asarray(dec_hidden, dtype=np.float32)
    enc_output = np.asarray(enc_output, dtype=np.float32)
    W1 = np.ascontiguousarray(np.asarray(W1, dtype=np.float32).astype(np.float16))
    W2 = np.ascontiguousarray(np.asarray(W2, dtype=np.float32).astype(np.float16))
    b12 = (np.asarray(b1, dtype=np.float32) + np.asarray(b2, dtype=np.float32))
    b12t = np.ascontiguousarray(b12.reshape(HC, P).T)
    Vt = np.ascontiguousarray(
        np.asarray(V, dtype=np.float32).reshape(UC, P).T.astype(np.float16)
    )
    # bV dropped: softmax is shift-invariant and score feeds only the softmax.
    in_maps = []
    for i in range(NCORES):
        sl = slice(i * BC, (i + 1) * BC)
        enc_sh = enc_output[sl].astype(np.float16)           # [BC, S, H]
        encT = np.ascontiguousarray(enc_sh.transpose(0, 2, 1))  # [BC, H, S]
        decT = np.ascontiguousarray(dec_hidden[sl].T.astype(np.float16))  # [H, BC]
        in_maps.append({
            "encT": encT,
            "encN": np.ascontiguousarray(enc_sh),
            "decT": decT,
            "W1": W1,
            "W2": W2,
            "Vt": Vt,
            "b12t": b12t,
        })
    return in_maps


def run_on_device(in_maps, trace=False):
    nc = _get_nc()
    return bass_utils.run_bass_kernel_spmd(
        nc, in_maps, core_ids=list(range(NCORES)), trace=trace
    )


def kernel(dec_hidden, enc_output, W1, b1, W2, b2, V, bV):
    in_maps = make_in_maps(dec_hidden, enc_output, W1, b1, W2, b2, V, bV)
    res = run_on_device(in_maps)
    ctx = np.concatenate([r["out_ctx"] for r in res.results], axis=0)
    w = np.concatenate([r["out_w"] for r in res.results], axis=0)
    return ctx.astype(np.float32), w[..., None].astype(np.float32)


# revision 23
# speedup vs baseline: 1.1489x; 1.1489x over previous
"""Bahdanau attention kernel for Trainium2 (8 NeuronCores, data-parallel over batch).

Computes, for inputs dec_hidden [B,H], enc_output [B,S,H], W1/W2 [H,U], b1/b2 [U],
V [U,1], bV [1]:
    q     = dec_hidden @ W1 + b1                      [B, U]
    k     = enc_output @ W2 + b2                      [B, S, U]
    score = tanh(q[:,None,:] + k) @ V + bV            [B, S, 1]
    w     = softmax(score, axis=1)                    [B, S, 1]
    ctx   = sum(w * enc_output, axis=1)               [B, H]
returns (ctx, w).

Sharding: batch is split 32 -> 8 cores x 4 examples. Weights replicated.
No collectives.

Device dataflow per core (4 examples), per example b and 512-wide s-block sb:
  - kT psum [u=128, s=512] = W2_tile.T @ encT_tile on TensorE (fp16 in, fp32 accum)
  - tanh(kT + (q+b1+b2)[u]) fused on ScalarE (bias = per-partition q column),
    written to SBUF as fp16
  - score psum [1, 512] = V_chunk.T @ tanh_tile accumulated over 8 u-chunks (PE)
  - ew = exp(score) on ScalarE, reading score psum directly, out fp16 (no max
    subtraction: |score| <= ||V||_1 ~ 26, exp is safe in fp32)
  - ew broadcast to all 128 partitions via rank-1 matmul ones[1,128].T @ ew[1,512]
  - ctx accumulation on VectorE: tensor_tensor_reduce multiplies encT tiles
    [h=128, s=512] by the broadcast weights and reduces along s, chaining the
    per-s-block partial into ctxT [128, HC]
Outputs are UNNORMALIZED (ew and ctx-sum); the host divides by sum(ew), which
is mathematically identical to softmax (softmax is shift-invariant so bV drops
out, and the normalizer cancels).

q is computed as qT [4, u] = decT.T @ W1 with 16 fat matmuls, then transposed
into [u=128, 4] chunks via PE transpose-mode, + (b1+b2) on VectorE.
"""

import numpy as np

import concourse.bass as bass
import concourse.mybir as mybir
import concourse.tile as tile
from concourse import bacc, bass_utils

# Problem shapes (hardcoded per the harness contract).
B, S, H, U = 32, 2048, 1024, 1024
NCORES = 8
BC = B // NCORES          # 4 examples per core
P = 128                   # partitions
HC = H // P               # 8 h-chunks
UC = U // P               # 8 u-chunks
NB = 512                  # free-dim block (one fp32 PSUM bank)
SB = S // NB              # 4 s-blocks
LAG = 8                   # deferred-emission queue depth (2 entries per u-group)

F32 = mybir.dt.float32
F16 = mybir.dt.float16

_COMPILED = None


def build_nc():
    """Build the single-core Bass program (SPMD across 8 cores)."""
    nc = bacc.Bacc(trn_type="TRN2")

    # ---- DRAM I/O (per-core shard) ----
    encT = nc.dram_tensor("encT", [BC, H, S], F16, kind="ExternalInput").ap()
    decT = nc.dram_tensor("decT", [H, BC], F16, kind="ExternalInput").ap()
    W1 = nc.dram_tensor("W1", [H, U], F16, kind="ExternalInput").ap()
    W2 = nc.dram_tensor("W2", [H, U], F16, kind="ExternalInput").ap()
    Vt = nc.dram_tensor("Vt", [P, UC], F16, kind="ExternalInput").ap()
    b12t = nc.dram_tensor("b12t", [P, UC], F32, kind="ExternalInput").ap()
    id4 = nc.dram_tensor("id4", [BC, BC], F16, kind="ExternalInput").ap()
    out_ew = nc.dram_tensor("out_ew", [BC, S], F16, kind="ExternalOutput").ap()
    out_ctxT = nc.dram_tensor("out_ctxT", [BC, P, HC], F32, kind="ExternalOutput").ap()

    TANH = mybir.ActivationFunctionType.Tanh
    EXP = mybir.ActivationFunctionType.Exp
    MULT = mybir.AluOpType.mult
    ADD = mybir.AluOpType.add
    XAX = mybir.AxisListType.X

    with tile.TileContext(nc) as tc:
        with (
            tc.tile_pool(name="const", bufs=1) as cpool,
            tc.tile_pool(name="encT", bufs=2) as etpool,
            tc.tile_pool(name="tanh", bufs=4) as thpool,
            tc.tile_pool(name="rows", bufs=2) as rpool,
            tc.tile_pool(name="wbc16", bufs=2) as wbpool,
            tc.tile_pool(name="tts", bufs=2) as ttpool,
            tc.tile_pool(name="mmps", bufs=5, space=bass.MemorySpace.PSUM) as mmps,
            tc.tile_pool(name="rowps", bufs=2, space=bass.MemorySpace.PSUM) as rowps,
            tc.tile_pool(name="wbps", bufs=1, space=bass.MemorySpace.PSUM) as wbps,
        ):
            # ---- weights: W2 first (gates the first main matmul) ----
            W2sb = cpool.tile([P, HC, U], F16)
            nc.sync.dma_start(W2sb[:], W2.rearrange("(c p) u -> p c u", p=P))

            enc_tiles = {}
            ew_rows = {}
            ctxT_tiles = {}

            def emit_encT_dma(b, et, sb):
                nc.sync.dma_start(
                    et[:, :, sb * NB:(sb + 1) * NB],
                    encT[b].rearrange("(c p) s -> p c s", p=P)[
                        :, :, sb * NB:(sb + 1) * NB
                    ],
                )

            # encT for example 0, s-block 0 next: first main matmul gate.
            et0 = etpool.tile([P, HC, S], F16, tag="encT")
            emit_encT_dma(0, et0, 0)
            enc_tiles[0] = et0

            # W1 next (gates the q phase, which slots in after a few groups)
            W1sb = cpool.tile([P, HC, U], F16)
            nc.sync.dma_start(W1sb[:], W1.rearrange("(c p) u -> p c u", p=P))

            # ---- remaining constants (small) ----
            decTsb = cpool.tile([P, HC, BC], F16)
            nc.sync.dma_start(decTsb[:], decT.rearrange("(c p) b -> p c b", p=P))
            Vtsb = cpool.tile([P, UC], F16)
            nc.sync.dma_start(Vtsb[:], Vt)
            b12sb = cpool.tile([P, UC], F32)
            nc.sync.dma_start(b12sb[:], b12t)
            ident4 = cpool.tile([BC, BC], F16)
            nc.sync.dma_start(ident4[:], id4)
            ones_row = cpool.tile([1, P], F16)
            nc.vector.memset(ones_row[:], 1.0)
            for sb in range(1, SB):
                emit_encT_dma(0, et0, sb)

            qbsb = cpool.tile([P, UC, BC], F32)

            def emit_q_phase():
                """qT = decT.T @ W1 (fat MMs), transpose to [u,4], add b1+b2."""
                qt16 = cpool.tile([BC, U], F16)
                for half in range(2):
                    qps = mmps.tile([P, NB], F32, tag="mm")
                    for hc in range(HC):
                        nc.tensor.matmul(
                            qps[:BC, :],
                            decTsb[:, hc, :],
                            W1sb[:, hc, half * NB:(half + 1) * NB],
                            start=(hc == 0),
                            stop=(hc == HC - 1),
                        )
                    nc.vector.tensor_copy(
                        qt16[:, half * NB:(half + 1) * NB], qps[:BC, :]
                    )
                for uc in range(UC):
                    tp = mmps.tile([P, NB], F16, tag="mm")
                    nc.tensor.transpose(
                        tp[:, :BC], qt16[:, uc * P:(uc + 1) * P], ident4[:]
                    )
                    nc.vector.tensor_scalar_add(
                        qbsb[:, uc, :], tp[:, :BC], b12sb[:, uc:uc + 1]
                    )

            # Deferred emission queue: PE ops gated by ACT/DVE results are
            # emitted LAG main-matmul groups later so PE never stalls.
            pending = []

            def drain(n_keep):
                while len(pending) > n_keep:
                    pending.pop(0)()

            def phase2(b):
                """Main matmul + tanh + score + per-s-block softmax/context."""
                if b in enc_tiles:
                    et = enc_tiles[b]
                else:
                    et = etpool.tile([P, HC, S], F16, tag="encT")
                    for sb in range(SB):
                        emit_encT_dma(b, et, sb)
                    enc_tiles[b] = et
                ew16 = rpool.tile([1, S], F16, tag="ew")
                ew_rows[b] = ew16
                ctxT = rpool.tile([P, HC], F32, tag="ctxT")
                ctxT_tiles[b] = ctxT
                pt = rpool.tile([P, HC, SB], F32, tag="pt")
                for sb in range(SB):
                    sps = rowps.tile([1, NB], F32, tag="row")
                    for uc in range(UC):
                        mps = mmps.tile([P, NB], F32, tag="mm")
                        for hc in range(HC):
                            nc.tensor.matmul(
                                mps[:],
                                W2sb[:, hc, uc * P:(uc + 1) * P],
                                et[:, hc, sb * NB:(sb + 1) * NB],
                                start=(hc == 0),
                                stop=(hc == HC - 1),
                            )
                        tt = thpool.tile([P, NB], F16, tag="tanh")

                        def tanh_op(tt=tt, mps=mps, uc=uc, b=b):
                            nc.scalar.activation(
                                tt[:], mps[:], TANH, bias=qbsb[:, uc, b:b + 1]
                            )

                        def score_mm(sps=sps, uc=uc, tt=tt):
                            nc.tensor.matmul(
                                sps[:],
                                Vtsb[:, uc:uc + 1],
                                tt[:],
                                start=(uc == 0),
                                stop=(uc == UC - 1),
                            )

                        pending.append(tanh_op)
                        pending.append(score_mm)

                        if uc == UC - 1:
                            def epilogue(b=b, sb=sb, sps=sps, ew16=ew16,
                                         ctxT=ctxT, pt=pt, et=et):
                                # ew = exp(score), fp16, straight from PSUM
                                nc.scalar.activation(
                                    ew16[:, sb * NB:(sb + 1) * NB], sps[:], EXP
                                )
                                # broadcast ew to 128 partitions: rank-1 matmul
                                wb = wbps.tile([P, NB], F32, tag="wb")
                                nc.tensor.matmul(
                                    wb[:], ones_row[:],
                                    ew16[:, sb * NB:(sb + 1) * NB],
                                    start=True, stop=True,
                                )
                                wb16 = wbpool.tile([P, NB], F16, tag="wb16")
                                nc.vector.tensor_copy(wb16[:], wb[:])
                                # ctx partials: sum_s wb*encT along s for all
                                # 8 h-chunks in one mult + one reduce
                                prod = ttpool.tile([P, HC, NB], F16, tag="tts")
                                wbap = wb16[:]
                                wbb = bass.AP(
                                    wbap.tensor, wbap.offset,
                                    [wbap.ap[0], [0, HC], wbap.ap[1]],
                                )
                                nc.vector.tensor_tensor(
                                    out=prod[:],
                                    in0=et[:, :, sb * NB:(sb + 1) * NB],
                                    in1=wbb, op=MULT,
                                )
                                nc.vector.tensor_reduce(
                                    out=pt[:, :, sb], in_=prod[:],
                                    axis=XAX, op=ADD,
                                )
                                if sb == SB - 1:
                                    nc.vector.tensor_reduce(
                                        out=ctxT[:], in_=pt[:],
                                        axis=XAX, op=ADD,
                                    )

                            pending.append(epilogue)

                        drain(LAG)
                        if b == 0 and sb == 0 and uc == 2:
                            emit_q_phase()

            def emit_outputs(b):
                nc.sync.dma_start(out_ew[b:b + 1, :], ew_rows[b][:])
                nc.sync.dma_start(out_ctxT[b], ctxT_tiles[b][:])

            phase2(0)
            phase2(1)
            drain(LAG)
            emit_outputs(0)
            phase2(2)
            emit_outputs(1)
            phase2(3)
            emit_outputs(2)
            drain(0)
            emit_outputs(3)

    nc.compile()
    return nc


def _get_nc():
    global _COMPILED
    if _COMPILED is None:
        _COMPILED = build_nc()
    return _COMPILED


def make_in_maps(dec_hidden, enc_output, W1, b1, W2, b2, V, bV):
    """Host-side sharding / layout prep. Returns list of 8 per-core input maps."""
    dec_hidden = np.asarray(dec_hidden, dtype=np.float32)
    enc_output = np.asarray(enc_output, dtype=np.float32)
    W1 = np.ascontiguousarray(np.asarray(W1, dtype=np.float32).astype(np.float16))
    W2 = np.ascontiguousarray(np.asarray(W2, dtype=np.float32).astype(np.float16))
    b12 = (np.asarray(b1, dtype=np.float32) + np.asarray(b2, dtype=np.float32))
    b12t = np.ascontiguousarray(b12.reshape(HC, P).T)
    Vt = np.ascontiguousarray(
        np.asarray(V, dtype=np.float32).reshape(UC, P).T.astype(np.float16)
    )
    # bV dropped: softmax is shift-invariant and score feeds only the softmax.
    in_maps = []
    for i in range(NCORES):
        sl = slice(i * BC, (i + 1) * BC)
        encT = np.ascontiguousarray(
            enc_output[sl].transpose(0, 2, 1).astype(np.float16))  # [BC, H, S]
        decT = np.ascontiguousarray(dec_hidden[sl].T.astype(np.float16))  # [H, BC]
        in_maps.append({
            "encT": encT,
            "decT": decT,
            "W1": W1,
            "W2": W2,
            "Vt": Vt,
            "b12t": b12t,
            "id4": np.eye(BC, dtype=np.float16),
        })
    return in_maps


def run_on_device(in_maps, trace=False):
    nc = _get_nc()
    return bass_utils.run_bass_kernel_spmd(
        nc, in_maps, core_ids=list(range(NCORES)), trace=trace
    )


def assemble_outputs(results):
    """Normalize on the host: softmax weights and context from raw exp sums."""
    ctx_list, w_list = [], []
    for r in results:
        ew = r["out_ew"].astype(np.float64)                 # [BC, S]
        tot = ew.sum(axis=1, keepdims=True)                 # [BC, 1]
        w_list.append((ew / tot).astype(np.float32))
        ctxT = r["out_ctxT"].astype(np.float64)             # [BC, P, HC]
        ctx = ctxT.transpose(0, 2, 1).reshape(BC, H)        # h = hc*P + p
        ctx_list.append((ctx / tot).astype(np.float32))
    ctx = np.concatenate(ctx_list, axis=0)
    w = np.concatenate(w_list, axis=0)[..., None]
    return ctx, w


def kernel(dec_hidden, enc_output, W1, b1, W2, b2, V, bV):
    in_maps = make_in_maps(dec_hidden, enc_output, W1, b1, W2, b2, V, bV)
    res = run_on_device(in_maps)
    return assemble_outputs(res.results)


# revision 32
# speedup vs baseline: 1.1736x; 1.0215x over previous
"""Bahdanau attention kernel for Trainium2 (8 NeuronCores, data-parallel over batch).

Computes, for inputs dec_hidden [B,H], enc_output [B,S,H], W1/W2 [H,U], b1/b2 [U],
V [U,1], bV [1]:
    q     = dec_hidden @ W1 + b1                      [B, U]
    k     = enc_output @ W2 + b2                      [B, S, U]
    score = tanh(q[:,None,:] + k) @ V + bV            [B, S, 1]
    w     = softmax(score, axis=1)                    [B, S, 1]
    ctx   = sum(w * enc_output, axis=1)               [B, H]
returns (ctx, w).

Sharding: batch is split 32 -> 8 cores x 4 examples. Weights replicated.
No collectives.

Device dataflow per core (4 examples), per example b and 512-wide s-block sb:
  - kT psum [u=128, s=512] = W2_tile.T @ encT_tile on TensorE (fp16 in, fp32 accum)
  - tanh(kT + (q+b1+b2)[u]) fused on ScalarE (bias = per-partition q column),
    written to SBUF as fp16
  - score psum [1, 512] = V_chunk.T @ tanh_tile accumulated over 8 u-chunks (PE)
  - ew = exp(score) on ScalarE, reading score psum directly, out fp16 (no max
    subtraction: |score| <= ||V||_1 ~ 26, exp is safe in fp32)
  - ew broadcast to all 128 partitions via rank-1 matmul ones[1,128].T @ ew[1,512]
  - ctx accumulation on VectorE: tensor_tensor_reduce multiplies encT tiles
    [h=128, s=512] by the broadcast weights and reduces along s, chaining the
    per-s-block partial into ctxT [128, HC]
Outputs are UNNORMALIZED (ew and ctx-sum); the host divides by sum(ew), which
is mathematically identical to softmax (softmax is shift-invariant so bV drops
out, and the normalizer cancels).

q is computed as qT [4, u] = decT.T @ W1 with 16 fat matmuls, then transposed
into [u=128, 4] chunks via PE transpose-mode, + (b1+b2) on VectorE.
"""

import numpy as np

import concourse.bass as bass
import concourse.mybir as mybir
import concourse.tile as tile
from concourse import bacc, bass_utils

# Problem shapes (hardcoded per the harness contract).
B, S, H, U = 32, 2048, 1024, 1024
NCORES = 8
BC = B // NCORES          # 4 examples per core
P = 128                   # partitions
HC = H // P               # 8 h-chunks
UC = U // P               # 8 u-chunks
NB = 512                  # free-dim block (one fp32 PSUM bank)
SB = S // NB              # 4 s-blocks
LAG = 8                   # deferred-emission queue depth (2 entries per u-group)

F32 = mybir.dt.float32
F16 = mybir.dt.float16

_COMPILED = None


def build_nc():
    """Build the single-core Bass program (SPMD across 8 cores)."""
    nc = bacc.Bacc(trn_type="TRN2")

    # ---- DRAM I/O (per-core shard), pre-arranged host-side so every DMA is
    # contiguous per partition ----
    encT = nc.dram_tensor("encT", [BC, SB, P, HC, NB], F16,
                          kind="ExternalInput").ap()
    decT = nc.dram_tensor("decT", [P, HC, BC], F16, kind="ExternalInput").ap()
    W1 = nc.dram_tensor("W1", [P, HC, U], F16, kind="ExternalInput").ap()
    W2 = nc.dram_tensor("W2", [P, HC, U], F16, kind="ExternalInput").ap()
    Vt = nc.dram_tensor("Vt", [P, UC], F16, kind="ExternalInput").ap()
    b12t = nc.dram_tensor("b12t", [P, UC], F32, kind="ExternalInput").ap()
    id4 = nc.dram_tensor("id4", [BC, BC], F16, kind="ExternalInput").ap()
    out_ew = nc.dram_tensor("out_ew", [BC, S], F16, kind="ExternalOutput").ap()
    out_ctxT = nc.dram_tensor("out_ctxT", [BC, P, HC], F32, kind="ExternalOutput").ap()

    TANH = mybir.ActivationFunctionType.Tanh
    EXP = mybir.ActivationFunctionType.Exp
    MULT = mybir.AluOpType.mult
    ADD = mybir.AluOpType.add
    XAX = mybir.AxisListType.X

    with tile.TileContext(nc) as tc:
        with (
            tc.tile_pool(name="const", bufs=1) as cpool,
            tc.tile_pool(name="encT", bufs=2) as etpool,
            tc.tile_pool(name="tanh", bufs=4) as thpool,
            tc.tile_pool(name="rows", bufs=2) as rpool,
            tc.tile_pool(name="wbc16", bufs=2) as wbpool,
            tc.tile_pool(name="tts", bufs=2) as ttpool,
            tc.tile_pool(name="mmps", bufs=5, space=bass.MemorySpace.PSUM) as mmps,
            tc.tile_pool(name="rowps", bufs=2, space=bass.MemorySpace.PSUM) as rowps,
            tc.tile_pool(name="wbps", bufs=1, space=bass.MemorySpace.PSUM) as wbps,
        ):
            # ---- weights: W2 first (gates the first main matmul) ----
            W2sb = cpool.tile([P, HC, U], F16)
            nc.sync.dma_start(W2sb[:], W2)

            enc_tiles = {}
            ew_rows = {}
            ctxT_tiles = {}

            def emit_encT_dma(b, et, sb):
                nc.sync.dma_start(et[:, sb], encT[b, sb])

            # encT for example 0, s-block 0 next: first main matmul gate.
            et0 = etpool.tile([P, SB, HC, NB], F16, tag="encT")
            emit_encT_dma(0, et0, 0)
            enc_tiles[0] = et0

            # W1 next (gates the q phase, which slots in after a few groups)
            W1sb = cpool.tile([P, HC, U], F16)
            nc.sync.dma_start(W1sb[:], W1)

            # ---- remaining constants (small) ----
            decTsb = cpool.tile([P, HC, BC], F16)
            nc.sync.dma_start(decTsb[:], decT)
            Vtsb = cpool.tile([P, UC], F16)
            nc.sync.dma_start(Vtsb[:], Vt)
            b12sb = cpool.tile([P, UC], F32)
            nc.sync.dma_start(b12sb[:], b12t)
            ident4 = cpool.tile([BC, BC], F16)
            nc.sync.dma_start(ident4[:], id4)
            ones_row = cpool.tile([1, P], F16)
            nc.vector.memset(ones_row[:], 1.0)
            for sb in range(1, SB):
                emit_encT_dma(0, et0, sb)

            qbsb = cpool.tile([P, UC, BC], F32)

            def emit_q_phase():
                """qT = decT.T @ W1 (fat MMs), transpose to [u,4], add b1+b2."""
                qt16 = cpool.tile([BC, U], F16)
                for half in range(2):
                    qps = mmps.tile([P, NB], F32, tag="mm")
                    for hc in range(HC):
                        nc.tensor.matmul(
                            qps[:BC, :],
                            decTsb[:, hc, :],
                            W1sb[:, hc, half * NB:(half + 1) * NB],
                            start=(hc == 0),
                            stop=(hc == HC - 1),
                        )
                    nc.vector.tensor_copy(
                        qt16[:, half * NB:(half + 1) * NB], qps[:BC, :]
                    )
                for uc in range(UC):
                    tp = mmps.tile([P, NB], F16, tag="mm")
                    nc.tensor.transpose(
                        tp[:, :BC], qt16[:, uc * P:(uc + 1) * P], ident4[:]
                    )
                    nc.vector.tensor_scalar_add(
                        qbsb[:, uc, :], tp[:, :BC], b12sb[:, uc:uc + 1]
                    )

            # Deferred emission queues: ops gated by other engines are emitted
            # 2-3 main-matmul groups later so PE never stalls. Tanh drains 2
            # groups back (frees its PSUM bank early); the score matmul drains
            # 3 groups back so its tanh input is already finished on ScalarE.
            tanh_q = []
            score_q = []

            def drain(tanh_keep, score_keep):
                while len(tanh_q) > tanh_keep:
                    tanh_q.pop(0)()
                while len(score_q) > score_keep:
                    score_q.pop(0)()

            def phase2(b):
                """Main matmul + tanh + score + per-s-block softmax/context."""
                if b in enc_tiles:
                    et = enc_tiles[b]
                else:
                    et = etpool.tile([P, SB, HC, NB], F16, tag="encT")
                    for sb in range(SB):
                        emit_encT_dma(b, et, sb)
                    enc_tiles[b] = et
                ew16 = rpool.tile([1, S], F16, tag="ew")
                ew_rows[b] = ew16
                ctxT = rpool.tile([P, HC], F32, tag="ctxT")
                ctxT_tiles[b] = ctxT
                pt = rpool.tile([P, HC, SB], F32, tag="pt")
                for sb in range(SB):
                    sps = rowps.tile([1, NB], F32, tag="row")
                    for uc in range(UC):
                        mps = mmps.tile([P, NB], F32, tag="mm")
                        for hc in range(HC):
                            nc.tensor.matmul(
                                mps[:],
                                W2sb[:, hc, uc * P:(uc + 1) * P],
                                et[:, sb, hc, :],
                                start=(hc == 0),
                                stop=(hc == HC - 1),
                            )
                        tt = thpool.tile([P, NB], F16, tag="tanh")

                        def tanh_op(tt=tt, mps=mps, uc=uc, b=b):
                            nc.scalar.activation(
                                tt[:], mps[:], TANH, bias=qbsb[:, uc, b:b + 1]
                            )

                        def score_mm(sps=sps, uc=uc, tt=tt):
                            nc.tensor.matmul(
                                sps[:],
                                Vtsb[:, uc:uc + 1],
                                tt[:],
                                start=(uc == 0),
                                stop=(uc == UC - 1),
                            )

                        tanh_q.append(tanh_op)
                        score_q.append(score_mm)

                        if uc == UC - 1:
                            def epilogue(b=b, sb=sb, sps=sps, ew16=ew16,
                                         ctxT=ctxT, pt=pt, et=et):
                                # ew = exp(score), fp16, straight from PSUM
                                nc.scalar.activation(
                                    ew16[:, sb * NB:(sb + 1) * NB], sps[:], EXP
                                )
                                # broadcast ew to 128 partitions: rank-1 matmul
                                wb = wbps.tile([P, NB], F32, tag="wb")
                                nc.tensor.matmul(
                                    wb[:], ones_row[:],
                                    ew16[:, sb * NB:(sb + 1) * NB],
                                    start=True, stop=True,
                                )
                                wb16 = wbpool.tile([P, NB], F16, tag="wb16")
                                nc.vector.tensor_copy(wb16[:], wb[:])
                                # ctx partials: sum_s wb*encT along s for all
                                # 8 h-chunks in one mult + one reduce
                                prod = ttpool.tile([P, HC, NB], F16, tag="tts")
                                wbap = wb16[:]
                                wbb = bass.AP(
                                    wbap.tensor, wbap.offset,
                                    [wbap.ap[0], [0, HC], wbap.ap[1]],
                                )
                                nc.vector.tensor_tensor(
                                    out=prod[:],
                                    in0=et[:, sb],
                                    in1=wbb, op=MULT,
                                )
                                nc.vector.tensor_reduce(
                                    out=pt[:, :, sb], in_=prod[:],
                                    axis=XAX, op=ADD,
                                )
                                if sb == SB - 1:
                                    nc.vector.tensor_reduce(
                                        out=ctxT[:], in_=pt[:],
                                        axis=XAX, op=ADD,
                                    )

                            score_q.append(epilogue)

                        drain(2, 3)
                        if b == 0 and sb == 0 and uc == 1:
                            emit_q_phase()

            def emit_outputs(b):
                nc.sync.dma_start(out_ew[b:b + 1, :], ew_rows[b][:])
                nc.sync.dma_start(out_ctxT[b], ctxT_tiles[b][:])

            phase2(0)
            phase2(1)
            emit_outputs(0)
            phase2(2)
            emit_outputs(1)
            phase2(3)
            emit_outputs(2)
            drain(0, 0)
            emit_outputs(3)

    nc.compile()
    return nc


def _get_nc():
    global _COMPILED
    if _COMPILED is None:
        _COMPILED = build_nc()
    return _COMPILED


def make_in_maps(dec_hidden, enc_output, W1, b1, W2, b2, V, bV):
    """Host-side sharding / layout prep. Returns list of 8 per-core input maps."""
    dec_hidden = np.asarray(dec_hidden, dtype=np.float32)
    enc_output = np.asarray(enc_output, dtype=np.float32)
    # weights in [p, hc, u] partition-major layout (h = hc*P + p)
    W1r = np.ascontiguousarray(
        np.asarray(W1, np.float32).astype(np.float16)
        .reshape(HC, P, U).transpose(1, 0, 2))
    W2r = np.ascontiguousarray(
        np.asarray(W2, np.float32).astype(np.float16)
        .reshape(HC, P, U).transpose(1, 0, 2))
    b12 = (np.asarray(b1, dtype=np.float32) + np.asarray(b2, dtype=np.float32))
    b12t = np.ascontiguousarray(b12.reshape(HC, P).T)
    Vt = np.ascontiguousarray(
        np.asarray(V, dtype=np.float32).reshape(UC, P).T.astype(np.float16)
    )
    # bV dropped: softmax is shift-invariant and score feeds only the softmax.
    in_maps = []
    for i in range(NCORES):
        sl = slice(i * BC, (i + 1) * BC)
        enc16 = enc_output[sl].astype(np.float16)            # [BC, S, H]
        # [b, sb, p, hc, j] = enc[b, sb*NB + j, hc*P + p]
        encT = np.ascontiguousarray(
            enc16.reshape(BC, SB, NB, HC, P).transpose(0, 1, 4, 3, 2))
        decT = np.ascontiguousarray(
            dec_hidden[sl].T.astype(np.float16)              # [H, BC]
            .reshape(HC, P, BC).transpose(1, 0, 2))          # [P, HC, BC]
        in_maps.append({
            "encT": encT,
            "decT": decT,
            "W1": W1r,
            "W2": W2r,
            "Vt": Vt,
            "b12t": b12t,
            "id4": np.eye(BC, dtype=np.float16),
        })
    return in_maps


def run_on_device(in_maps, trace=False):
    nc = _get_nc()
    return bass_utils.run_bass_kernel_spmd(
        nc, in_maps, core_ids=list(range(NCORES)), trace=trace
    )


def assemble_outputs(results):
    """Normalize on the host: softmax weights and context from raw exp sums."""
    ctx_list, w_list = [], []
    for r in results:
        ew = r["out_ew"].astype(np.float64)                 # [BC, S]
        tot = ew.sum(axis=1, keepdims=True)                 # [BC, 1]
        w_list.append((ew / tot).astype(np.float32))
        ctxT = r["out_ctxT"].astype(np.float64)             # [BC, P, HC]
        ctx = ctxT.transpose(0, 2, 1).reshape(BC, H)        # h = hc*P + p
        ctx_list.append((ctx / tot).astype(np.float32))
    ctx = np.concatenate(ctx_list, axis=0)
    w = np.concatenate(w_list, axis=0)[..., None]
    return ctx, w


def kernel(dec_hidden, enc_output, W1, b1, W2, b2, V, bV):
    in_maps = make_in_maps(dec_hidden, enc_output, W1, b1, W2, b2, V, bV)
    res = run_on_device(in_maps)
    return assemble_outputs(res.results)
